# revision 10
# baseline (speedup 1.0000x reference)
"""Causal self-attention (B=2, T=2048, C=768, H=12) on 8 TRN2 NeuronCores.

Sharding: data-parallel over B (cores 0-3 -> b=0, cores 4-7 -> b=1), tensor
parallel over heads (3 heads per core). Each core computes q/k/v projections
for its 3 heads, causal attention, and a partial output projection; the host
sums the 4 partials per batch element and adds the output/v biases.

Attention is computed transposed: S^T[tk, tq] = K Q^T so that the softmax
denominator comes out of the ones-augmented AV matmul (V | 1) as row 64 of
the [65, 1024] PSUM accumulator; exp runs on the scalar engine straight out
of PSUM with 1/sqrt(d) folded into the activation scale.

Query columns are processed in two 1024-wide chunks.  Per chunk, heads A,B
run a merged sweep over k-tiles (their K=64 S-matmuls packed on PE row
strips 0-63 / 64-127 so they execute concurrently); head C runs a second
sweep with its S-matmul halves strip-paired via duplicated kC/qC copies.
One exp activation covers a whole [128, <=1024] S tile spanning two PSUM
banks, amortizing the ~200ns ACTIVATE overhead.  Diagonal tiles compute
only their causally valid column range; the 128x128 triangle mask multiply
runs on the otherwise-idle gpsimd engine, off the DVE/ACT critical path.

The attention sweeps are ACT(exp)-bound, so q/k/v projections for the next
chunk and the out-projection of the previous chunk are injected between
sweep tile-steps to fill PE idle time.  The w2c (head C) out-proj matmuls
are strip-paired inside each [128,1024] group.  Partial outputs leave the
device as bf16, halving the output DMA.

The v bias never touches the device: softmax rows sum to 1, so its
contribution is the constant vector out_w @ qkv_b[2C:], added on the host.
"""

import numpy as np
import ml_dtypes
from contextlib import ExitStack

import concourse.bass as bass
import concourse.tile as tile
from concourse import bacc, mybir
from concourse.bass_utils import run_bass_kernel_spmd

BF16 = mybir.dt.bfloat16
F32 = mybir.dt.float32
AF = mybir.ActivationFunctionType

B, T, C, H, D = 2, 2048, 768, 12, 64
HPC = 3          # heads per core
NCORES = 8
CC = C // 128    # 6 contraction chunks
NT = T // 128    # 16 tk tiles
NJ = T // 1024   # 2 tq chunks
CW = 1024        # chunk width
VW = D + 1       # 65: v columns + ones column
SCALE = float(D) ** -0.5

_cache = {}
DEBUG = False


def _build_program():
    nc = bacc.Bacc("TRN2", target_bir_lowering=False, debug=False,
                   enable_asserts=False, num_devices=NCORES)

    xt_d = nc.dram_tensor("xt_s", [128, CC * T], BF16, kind="ExternalInput").ap()
    wqk_d = nc.dram_tensor("wqk_s", [128, CC * 384], BF16, kind="ExternalInput").ap()
    wv_d = nc.dram_tensor("wv_s", [128, CC * 192], BF16, kind="ExternalInput").ap()
    bqk_d = nc.dram_tensor("bqk_s", [128, 3], F32, kind="ExternalInput").ap()
    w2ab_d = nc.dram_tensor("w2ab_s", [128, C], BF16, kind="ExternalInput").ap()
    w2c_d = nc.dram_tensor("w2c_s", [128, C], BF16, kind="ExternalInput").ap()
    tri_d = nc.dram_tensor("tri_s", [128, 128], BF16, kind="ExternalInput").ap()
    outp_d = nc.dram_tensor("outp", [128, CC * T], BF16, kind="ExternalOutput").ap()

    with tile.TileContext(nc) as tc, ExitStack() as ctx:
        const = ctx.enter_context(tc.tile_pool(name="const", bufs=1))
        big = ctx.enter_context(tc.tile_pool(name="big", bufs=1))
        spool = ctx.enter_context(tc.tile_pool(name="spool", bufs=2, space="PSUM"))
        avpool = ctx.enter_context(tc.tile_pool(name="avpool", bufs=2, space="PSUM"))
        ppool = ctx.enter_context(tc.tile_pool(name="ppool", bufs=4))
        small = ctx.enter_context(tc.tile_pool(name="small", bufs=6))

        # warm the ACT exp table while DMAs are in flight
        warm = small.tile([1, 16], F32, tag="warm")
        nc.gpsimd.memset(warm[:], 0.0)
        warm2 = small.tile([1, 16], F32, tag="warm")
        nc.scalar.activation(warm2[:], warm[:], AF.Exp)

        # PE warm-up matmuls: reach HAM K=8/8 during the input DMA wait
        wz = const.tile([128, 512], BF16, name="wz")
        nc.gpsimd.memset(wz[:], 0.0)
        wps = spool.tile([128, CW], F32, tag="sp", name="wps")
        for _ in range(8):
            nc.tensor.matmul(wps[:, 0:512], wz[:, 0:128], wz[:],
                             start=True, stop=True)

        # ---- load constants/inputs (order matters: earliest consumers first)
        wqk = const.tile([128, CC * 384], BF16)
        nc.sync.dma_start(wqk[:], wqk_d[:])
        bqk = const.tile([128, 3], F32)
        nc.sync.dma_start(bqk[:], bqk_d[:])
        # xt loaded chunk-major so the first qk group only waits for chunk 0
        xt = const.tile([128, CC * T], BF16)
        for cj in range(NJ):
            for kc in range(CC):
                sl = slice(kc * T + cj * CW, kc * T + (cj + 1) * CW)
                nc.sync.dma_start(xt[:, sl], xt_d[:, sl])
            if cj == 0:
                wv = const.tile([128, CC * 192], BF16)
                nc.sync.dma_start(wv[:], wv_d[:])
                tri = const.tile([128, 128], BF16)
                nc.sync.dma_start(tri[:], tri_d[:])
                w2ab = const.tile([128, C], BF16)
                nc.sync.dma_start(w2ab[:], w2ab_d[:])
                w2c2 = const.tile([128, C], BF16)
                nc.sync.dma_start(w2c2[:], w2c_d[:])

        # ---- persistent intermediates
        qt1 = big.tile([128, T], BF16)    # qA (p 0-63) | qB (p 64-127), [d, t]
        kt1 = big.tile([128, T], BF16)    # kA | kB
        qkt2 = big.tile([128, T], BF16)   # qC | kC
        kt2 = big.tile([64, T], BF16)     # kC shifted to partitions 0-63
        qdup = big.tile([128, CW], BF16)  # qC (current chunk) at p 64-127
        vbuf = big.tile([128, NT * HPC * VW], BF16)
        ot_ab = big.tile([128, T], BF16)  # O.T heads A,B (out-proj rhs)
        ot_c2 = big.tile([128, T], BF16)  # O.T head C, duplicated lo|hi

        nc.gpsimd.memset(vbuf[:], 1.0)

        def v_ap(h, i):
            off = i * HPC * VW + h * VW
            return vbuf[:, off:off + VW]

        qk_dest = [qt1, kt1, qkt2]

        def emit_qk_part(jt, c, half, cell):
            # half 0 allocates the psum tile + kc 0-2; half 1 finishes + bias
            cs = c * CW
            if half == 0:
                cell["ps"] = spool.tile([128, CW], F32, tag="sp",
                                        name=f"qk_{jt}_{c}")
            ps = cell["ps"]
            rng = range(0, 3) if half == 0 else range(3, 6)
            for kc in rng:
                for hw_ in (0, 512):
                    nc.tensor.matmul(
                        ps[:, hw_:hw_ + 512],
                        wqk[:, kc * 384 + jt * 128: kc * 384 + (jt + 1) * 128],
                        xt[:, kc * T + cs + hw_: kc * T + cs + hw_ + 512],
                        start=(kc == 0), stop=(kc == CC - 1),
                    )
            if half == 1:
                nc.vector.tensor_scalar_add(
                    qk_dest[jt][:, cs:cs + CW], ps[:], bqk[:, jt:jt + 1])

        def emit_shift(c):
            cs = c * CW
            nc.sync.dma_start(kt2[:, cs:cs + CW], qkt2[64:128, cs:cs + CW])

        def emit_qdup(c):
            cs = c * CW
            nc.sync.dma_start(qdup[64:128, :], qkt2[0:64, cs:cs + CW])

        def emit_v_group(ti):
            ps = spool.tile([128, 192], F32, tag="sp", name=f"v_{ti}")
            for kc in range(CC):
                nc.tensor.matmul(
                    ps[:],
                    xt[:, kc * T + ti * 128: kc * T + (ti + 1) * 128],
                    wv[:, kc * 192:(kc + 1) * 192],
                    start=(kc == 0), stop=(kc == CC - 1),
                )
            dst = vbuf[:, ti * HPC * VW:(ti + 1) * HPC * VW]
            dst = dst.rearrange("p (h x) -> p h x", h=HPC)[:, :, 0:D]
            nc.vector.tensor_copy(
                dst, ps[:].rearrange("p (h x) -> p h x", h=HPC))

        def emit_op_group(jt, c):
            cs = c * CW
            jsl = slice(jt * 128, (jt + 1) * 128)
            ps = spool.tile([128, CW], F32, tag="sp", name=f"op_{jt}_{c}")
            for hw_ in (0, 512):
                nc.tensor.matmul(ps[:, hw_:hw_ + 512], w2ab[:, jsl],
                                 ot_ab[:, cs + hw_:cs + hw_ + 512],
                                 start=True, stop=False, skip_group_check=True)
            # head-C contribution strip-paired: lo does cols 0-511, hi 512-1023
            nc.tensor.matmul(ps[:, 0:512], w2c2[0:64, jsl],
                             ot_c2[0:64, cs:cs + 512],
                             start=False, stop=True, skip_group_check=True)
            nc.tensor.matmul(ps[:, 512:CW], w2c2[64:128, jsl],
                             ot_c2[64:128, cs + 512:cs + CW],
                             start=False, stop=True, skip_group_check=True)
            ob = small.tile([128, CW], BF16, tag="ob", name=f"ob_{jt}_{c}")
            nc.vector.tensor_copy(ob[:], ps[:])
            nc.sync.dma_start(outp_d[:, jt * T + cs: jt * T + cs + CW], ob[:])

        def pieces(c0):
            if c0 < 512:
                return [(c0, 512), (512, CW)]
            return [(c0, CW)]

        inj = []

        def pop_inj(n=1):
            for _ in range(min(n, len(inj))):
                inj.pop(0)()

        def normalize(h, c, av):
            cs = c * CW
            den = small.tile([1, CW], F32, tag="den", name=f"den_{h}_{c}")
            nc.vector.tensor_copy(den[:], av[D:VW, :])
            recip = small.tile([1, CW], F32, tag="recip", name=f"rc_{h}_{c}")
            # custom-DVE ops read garbage from PSUM; SBUF source only
            nc.vector.reciprocal_approx_fast(recip[:], den[:])
            rb = small.tile([64, CW], F32, tag="rb", name=f"rb_{h}_{c}")
            nc.gpsimd.partition_broadcast(rb[:], recip[:])
            if h == 0:
                dst = ot_ab[0:64, cs:cs + CW]
                nc.vector.tensor_mul(dst, av[0:D, :], rb[:])
            elif h == 1:
                otb = small.tile([64, CW], BF16, tag="otb", name=f"otb_{c}")
                nc.vector.tensor_mul(otb[:], av[0:D, :], rb[:])
                nc.sync.dma_start(ot_ab[64:128, cs:cs + CW], otb[:])
            else:
                dst = ot_c2[0:64, cs:cs + CW]
                nc.vector.tensor_mul(dst, av[0:D, :], rb[:])
                nc.sync.dma_start(ot_c2[64:128, cs:cs + CW], dst)

        def sweep_ab(c, vsched=None):
            # vsched: {step: ti} v-projection groups that MUST be emitted by
            # that step (AV of tile ti reads vbuf[ti] in program order)
            cs = c * CW
            ntiles = 8 * c + 8
            av = {h: avpool.tile([VW, CW], F32, tag="av",
                                 name=f"av_{h}_{c}") for h in (0, 1)}
            prev = []   # (h, i, pt_ap, c0) from previous tile-step

            def flush_av(last=False):
                for (h, i, pt_ap, c0) in prev:
                    for (p0, p1) in pieces(c0):
                        nc.tensor.matmul(
                            av[h][:, p0:p1], v_ap(h, i),
                            pt_ap[:, p0 - c0:p1 - c0],
                            start=(i == 0), stop=last,
                            skip_group_check=True,
                        )
                prev.clear()

            for i in range(ntiles):
                it = slice(i * 128, (i + 1) * 128)
                c0 = max(0, i * 128 - cs)
                w = CW - c0
                diag = i * 128 >= cs
                sp = {h: spool.tile([128, CW], F32, tag="sp",
                                    name=f"s_{h}_{c}_{i}") for h in (0, 1)}
                for (p0, p1) in pieces(c0):
                    for h in (0, 1):
                        hp = slice(0, 64) if h == 0 else slice(64, 128)
                        nc.tensor.matmul(
                            sp[h][:, p0:p1], kt1[hp, it],
                            qt1[hp, cs + p0:cs + p1],
                            start=True, stop=True)
                flush_av()
                new = []
                for h in (0, 1):
                    pt = ppool.tile([128, w], BF16, tag="pt",
                                    name=f"pt_{h}_{c}_{i}")
                    nc.scalar.activation(pt[:], sp[h][:, c0:CW], AF.Exp,
                                         scale=SCALE)
                    if diag:
                        nc.vector.tensor_mul(pt[:, 0:128], pt[:, 0:128],
                                             tri[:])
                    new.append((h, i, pt[:], c0))
                prev.extend(new)
                if vsched and i in vsched:
                    emit_v_group(vsched[i])
                else:
                    pop_inj()
            flush_av(last=True)
            for h in (0, 1):
                normalize(h, c, av[h])

        def sweep_c(c):
            cs = c * CW
            ntiles = 8 * c + 8
            av = avpool.tile([VW, CW], F32, tag="av", name=f"av_2_{c}")
            prev = []

            def flush_av(last=False):
                for (i, pt_ap, c0) in prev:
                    for (p0, p1) in pieces(c0):
                        nc.tensor.matmul(
                            av[:, p0:p1], v_ap(2, i),
                            pt_ap[:, p0 - c0:p1 - c0],
                            start=(i == 0), stop=last,
                            skip_group_check=True,
                        )
                prev.clear()

            for i in range(ntiles):
                it = slice(i * 128, (i + 1) * 128)
                c0 = max(0, i * 128 - cs)
                w = CW - c0
                diag = i * 128 >= cs
                sp = spool.tile([128, CW], F32, tag="sp", name=f"s_2_{c}_{i}")
                pcs = pieces(c0)
                for pi, (p0, p1) in enumerate(pcs):
                    # strip-pair the two halves; lone pieces alternate by i
                    lo = (pi == 0) if len(pcs) == 2 else (i % 2 == 0)
                    if lo:
                        nc.tensor.matmul(
                            sp[:, p0:p1], kt2[0:64, it],
                            qkt2[0:64, cs + p0:cs + p1],
                            start=True, stop=True)
                    else:
                        nc.tensor.matmul(
                            sp[:, p0:p1], qkt2[64:128, it],
                            qdup[64:128, p0:p1],
                            start=True, stop=True)
                flush_av()
                pt = ppool.tile([128, w], BF16, tag="pt", name=f"pt_2_{c}_{i}")
                nc.scalar.activation(pt[:], sp[:, c0:CW], AF.Exp, scale=SCALE)
                if diag:
                    nc.vector.tensor_mul(pt[:, 0:128], pt[:, 0:128], tri[:])
                prev.append((i, pt[:], c0))
                pop_inj()
            flush_av(last=True)
            normalize(2, c, av)

        # prologue: projections needed by chunk 0
        cell = {}
        for jt in range(3):
            emit_qk_part(jt, 0, 0, cell)
            emit_qk_part(jt, 0, 1, cell)
        emit_shift(0)
        emit_qdup(0)
        emit_v_group(0)
        emit_v_group(1)

        # injection queue: work for later phases, one item per tile-step.
        # v groups are NOT queued: they ride a per-sweep schedule so that
        # vbuf[ti] is always written before the AV matmul that reads it.
        cell1 = [{}, {}, {}]
        inj += [lambda h=h: emit_qk_part(0, 1, h, cell1[0]) for h in (0, 1)]
        inj += [lambda h=h: emit_qk_part(1, 1, h, cell1[1]) for h in (0, 1)]
        inj += [lambda h=h: emit_qk_part(2, 1, h, cell1[2]) for h in (0, 1)]

        # chunk-0 AB sweep emits v2..v7 (AV of tile i reads v(i) at step i+1)
        sweep_ab(0, vsched={0: 2, 1: 3, 2: 4, 3: 5, 4: 6, 5: 7})
        sweep_c(0)
        inj += [lambda jt=jt: emit_op_group(jt, 0) for jt in range(CC)]
        emit_shift(1)
        emit_qdup(1)
        sweep_ab(1, vsched={i: i + 8 for i in range(8)})
        sweep_c(1)
        pop_inj(len(inj))
        for jt in range(CC):
            emit_op_group(jt, 1)

        if DEBUG:
            for nm, tl in [("dbg_qt1", qt1), ("dbg_kt1", kt1),
                           ("dbg_qkt2", qkt2), ("dbg_kt2", kt2),
                           ("dbg_vbuf", vbuf), ("dbg_otab", ot_ab),
                           ("dbg_otc2", ot_c2)]:
                shp = [tl.shape[0], tl.shape[1]]
                dd = nc.dram_tensor(nm, shp, BF16, kind="ExternalOutput").ap()
                nc.sync.dma_start(dd[:], tl[:])

    nc.compile()
    return nc


def _prep_in_maps(x, qkv_w, qkv_b, out_w):
    bf = ml_dtypes.bfloat16
    in_maps = []

    # causal triangle for diagonal 128x128 blocks: keep when f >= p
    p = np.arange(128)[:, None]
    f = np.arange(128)[None, :]
    tri_s = (f >= p).astype(bf)

    for c in range(NCORES):
        b = c // 4
        h0 = (c % 4) * HPC
        hs = [h0, h0 + 1, h0 + 2]

        xT = np.ascontiguousarray(x[b].T.astype(np.float32))  # [768, 2048]
        xt_s = xT.reshape(CC, 128, T).transpose(1, 0, 2).reshape(128, CC * T)

        qr = lambda h: qkv_w[h * D:(h + 1) * D]
        kr = lambda h: qkv_w[C + h * D: C + (h + 1) * D]
        vr = lambda h: qkv_w[2 * C + h * D: 2 * C + (h + 1) * D]
        qb = lambda h: qkv_b[h * D:(h + 1) * D]
        kb = lambda h: qkv_b[C + h * D: C + (h + 1) * D]

        wqk = np.concatenate([qr(hs[0]), qr(hs[1]), kr(hs[0]), kr(hs[1]),
                              qr(hs[2]), kr(hs[2])], axis=0)  # [384, 768]
        wqk_s = np.ascontiguousarray(wqk.T).reshape(CC, 128, 384) \
            .transpose(1, 0, 2).reshape(128, CC * 384)
        wv_ = np.concatenate([vr(h) for h in hs], axis=0)      # [192, 768]
        wv_s = np.ascontiguousarray(wv_.T).reshape(CC, 128, 192) \
            .transpose(1, 0, 2).reshape(128, CC * 192)

        bqk = np.concatenate([qb(hs[0]), qb(hs[1]), kb(hs[0]), kb(hs[1]),
                              qb(hs[2]), kb(hs[2])])
        bqk_s = np.ascontiguousarray(bqk.reshape(3, 128).T).astype(np.float32)

        ch_ab = np.r_[hs[0] * D:(hs[0] + 1) * D, hs[1] * D:(hs[1] + 1) * D]
        ch_c = np.r_[hs[2] * D:(hs[2] + 1) * D]
        w2ab_s = np.ascontiguousarray(out_w[:, ch_ab].T)  # [128, 768]
        w2c_1 = np.ascontiguousarray(out_w[:, ch_c].T)    # [64, 768]
        w2c_s = np.concatenate([w2c_1, w2c_1], axis=0)    # [128, 768] lo|hi

        in_maps.append({
            "xt_s": np.ascontiguousarray(xt_s).astype(bf),
            "wqk_s": np.ascontiguousarray(wqk_s).astype(bf),
            "wv_s": np.ascontiguousarray(wv_s).astype(bf),
            "bqk_s": bqk_s,
            "w2ab_s": w2ab_s.astype(bf),
            "w2c_s": w2c_s.astype(bf),
            "tri_s": tri_s,
        })
    return in_maps


def _assemble(results, qkv_b, out_w, out_b):
    out = np.zeros((B, T, C), dtype=np.float32)
    for c in range(NCORES):
        b = c // 4
        outp = results[c]["outp"].astype(np.float32)  # [128, CC*T] bf16
        outT = outp.reshape(128, CC, T).transpose(1, 0, 2).reshape(C, T)
        out[b] += outT.T
    # v-bias contribution (softmax rows sum to 1) + output bias
    const = out_w.astype(np.float32) @ qkv_b[2 * C:].astype(np.float32) \
        + out_b.astype(np.float32)
    out += const[None, None, :]
    return out


def run(x, qkv_w, qkv_b, out_w, out_b, trace=False, tmpdir=None):
    if "nc" not in _cache:
        _cache["nc"] = _build_program()
    nc = _cache["nc"]
    x = np.asarray(x, dtype=np.float32)
    qkv_w = np.asarray(qkv_w, dtype=np.float32)
    qkv_b = np.asarray(qkv_b, dtype=np.float32)
    out_w = np.asarray(out_w, dtype=np.float32)
    out_b = np.asarray(out_b, dtype=np.float32)
    in_maps = _prep_in_maps(x, qkv_w, qkv_b, out_w)
    res = run_bass_kernel_spmd(nc, in_maps, list(range(NCORES)), trace=trace,
                               tmpdir=tmpdir)
    out = _assemble(res.results, qkv_b, out_w, out_b)
    return out, res


def kernel(x, qkv_w, qkv_b, out_w, out_b):
    out, _ = run(x, qkv_w, qkv_b, out_w, out_b, trace=False)
    return out
